# revision 7
# baseline (speedup 1.0000x reference)
"""AdaptiveTokenMixer Trainium2 kernel (8 NeuronCores, pure data parallel).

Per-core algorithm (one batch element per core), pipelined per 5-block group:
  1. alpha stage runs in [105, 320] layout (partition q = (third h, block b),
     free = (pos i, tap p)); sliding windows are overlapping-stride AP reads
     of per-block dt/valid rows -- no PE transposes. Masked temporal-decay
     softmax blended with host-precomputed (b/(1-b))*softmax(w), renormalized.
  2. alpha (bf16) is scattered to a DRAM scratch with a skewed access pattern
     forming banded W^T[m, k] = alpha[n0+m, k-m] per 120-position block.
  3. W[k, m] is loaded back per block via the DMA-transpose XBAR (no PE
     transpose), then one 128x120 @ 128x256 bf16 matmul per block realizes
     the K-tap mixing exactly (PSUM f32).
  4. PSUM evicted to bf16 staging (DVE/ACT/Pool round-robin), stores per
     5-block group overlap the remaining matmuls.

Self-contained: hardcodes shapes for B=8, N=4096, d=256, K=8.
"""
import numpy as np
import ml_dtypes

import concourse.bass as bass
import concourse.bacc as bacc
import concourse.mybir as mybir
from concourse import tile
from concourse.bass_utils import run_bass_kernel_spmd

B, N, D, K = 8, 4096, 256, 8
BLK = 120                      # output positions per block
NB = (N + BLK - 1) // BLK      # 35 blocks -> covers 4200 positions
NOUT = NB * BLK                # 4200 rows in padded device output
NPAD = 4224                    # padded input length
KW = 128                       # k-window (contraction) per block
WBLK = KW * KW                 # W scratch elements per block
GB = 5                         # blocks per pipeline group
G = NB // GB                   # 7 groups
NH = 3                         # thirds per block in alpha layout
IH = BLK // NH                 # 40 positions per third
QP = NH * NB                   # 105 partitions used in alpha stage
FA = IH * K                    # 320 free elements per alpha partition
ROWW = IH + K                  # 48: dt/vf row width per third

_CACHE = {}


def _build():
    nc = bacc.Bacc("TRN2", target_bir_lowering=False, debug=False,
                   num_devices=B)
    f32 = mybir.dt.float32
    bf16 = mybir.dt.bfloat16

    x_t = nc.dram_tensor("x", [NPAD, D], bf16, kind="ExternalInput")
    dt_t = nc.dram_tensor("dt", [NPAD], f32, kind="ExternalInput")
    vf_t = nc.dram_tensor("vf", [NPAD], f32, kind="ExternalInput")
    bwsm_t = nc.dram_tensor("bwsm", [128, K], f32, kind="ExternalInput")
    wz_t = nc.dram_tensor("wz", [NB * WBLK], bf16, kind="ExternalInput")
    out_t = nc.dram_tensor("out", [NOUT, D], bf16, kind="ExternalOutput")

    def shift(t, p0):  # [QP, (i, p)] read of t[q, i+p] (overlapping strides)
        return bass.AP(t.tensor, t.offset + p0, [t.ap[0], [1, IH], [1, K]])

    def base(t):  # [QP, (i, p-rep)] read of t[q, i]
        return bass.AP(t.tensor, t.offset, [t.ap[0], [1, IH], [0, K]])

    def pb(t):  # [QP, 320] tile -> [QP, i, p] view (p innermost, for reduce)
        return bass.AP(t.tensor, t.offset, [t.ap[0], [K, IH], [1, K]])

    def exp_i(t):  # [QP, IH] tile -> [QP, (i, p-rep)]
        return bass.AP(t.tensor, t.offset, [t.ap[0], [1, IH], [0, K]])

    def bw_rep(t):  # [QP, K] tile -> [QP, (i-rep, p)]
        return bass.AP(t.tensor, t.offset, [t.ap[0], [0, IH], [1, K]])

    with tile.TileContext(nc) as tc:
        with tc.tile_pool(name="alph", bufs=1) as apool, \
             tc.tile_pool(name="big", bufs=1) as bpool, \
             tc.tile_pool(name="w", bufs=4) as wpool, \
             tc.tile_pool(name="ps", bufs=4, space="PSUM") as pspool:

            # ---- input loads ----
            bwsm = apool.tile([QP, K], f32)
            nc.scalar.dma_start(bwsm[:], bass.AP(bwsm_t, 0, [[K, QP], [1, K]]))
            dt_rows = apool.tile([QP, ROWW], f32)
            vf_rows = apool.tile([QP, ROWW], f32)
            for h in range(NH):
                for src_t, dst in ((dt_t, dt_rows), (vf_t, vf_rows)):
                    nc.scalar.dma_start(
                        dst[h * NB:(h + 1) * NB, :],
                        bass.AP(src_t, h * IH, [[BLK, NB], [1, ROWW]]))
            # x windows per group: x_all[i, b, d] = x[b*120+i, d]
            x_all = bpool.tile([128, NB, D], bf16)
            for g in range(G):
                nc.sync.dma_start(
                    x_all[:, g * GB:(g + 1) * GB, :],
                    bass.AP(x_t, g * GB * BLK * D,
                            [[D, 128], [BLK * D, GB], [1, D]]))

            # ---- alpha stage on [QP, FA] ----
            td = apool.tile([QP, FA], f32)
            nc.vector.tensor_tensor(td[:], shift(dt_rows, 0), base(dt_rows),
                                    mybir.AluOpType.subtract)
            nc.gpsimd.tensor_scalar(td[:], td[:], 0.0, None,
                                    mybir.AluOpType.max)
            cv = apool.tile([QP, FA], f32)
            nc.gpsimd.tensor_tensor(cv[:], shift(vf_rows, 0), base(vf_rows),
                                    mybir.AluOpType.mult)
            e2 = apool.tile([QP, FA], f32)
            nc.scalar.activation(e2[:], td[:],
                                 mybir.ActivationFunctionType.Exp,
                                 scale=-1.0)
            e = apool.tile([QP, FA], f32)
            nc.vector.tensor_tensor(e[:], e2[:], cv[:], mybir.AluOpType.mult)
            bwcv = apool.tile([QP, FA], f32)
            nc.gpsimd.tensor_tensor(bwcv[:], cv[:], bw_rep(bwsm[:, :]),
                                    mybir.AluOpType.mult)
            s = apool.tile([QP, IH], f32)
            nc.vector.tensor_reduce(s[:], pb(e), mybir.AxisListType.X,
                                    mybir.AluOpType.add)
            nc.vector.tensor_scalar(s[:], s[:], 1e-30, None,
                                    mybir.AluOpType.max)
            rcp = apool.tile([QP, IH], f32)
            nc.vector.reciprocal(rcp[:], s[:])
            au = apool.tile([QP, FA], f32)
            nc.vector.tensor_tensor(au[:], e[:], exp_i(rcp[:, :]),
                                    mybir.AluOpType.mult)
            nc.vector.tensor_tensor(au[:], au[:], bwcv[:],
                                    mybir.AluOpType.add)
            sa = apool.tile([QP, IH], f32)
            nc.vector.tensor_reduce(sa[:], pb(au), mybir.AxisListType.X,
                                    mybir.AluOpType.add)
            nc.vector.tensor_scalar(sa[:], sa[:], 1e-8, None,
                                    mybir.AluOpType.max)
            r = apool.tile([QP, IH], f32)
            nc.vector.reciprocal(r[:], sa[:])
            nc.vector.tensor_tensor(
                r[:], r[:],
                bass.AP(vf_rows.tensor, vf_rows.offset,
                        [vf_rows.ap[0], [1, IH]]),
                mybir.AluOpType.mult)
            af = apool.tile([QP, FA], bf16)
            nc.vector.tensor_tensor(af[:], au[:], exp_i(r[:, :]),
                                    mybir.AluOpType.mult)

            # ---- pipeline per group: skew -> W via xbar -> matmul -> evict
            out_all = bpool.tile([128, NB, D], bf16)
            evict = [nc.vector.tensor_copy, nc.scalar.copy]
            for g in range(G):
                # skewed W^T write (one DMA per third):
                #   wz[b*WBLK + (h*IH+i)*129 + p] = af[h*35+b, i*8+p]
                for h in range(NH):
                    v = af[h * NB + g * GB: h * NB + (g + 1) * GB]
                    nc.scalar.dma_start(
                        bass.AP(wz_t, g * GB * WBLK + h * IH * (KW + 1),
                                [[WBLK, GB], [KW + 1, IH], [1, K]]),
                        bass.AP(v.tensor, v.offset, [v.ap[0], [K, IH], [1, K]]))
                for j in range(GB):
                    b = g * GB + j
                    # W[k, m] directly via DMA-transpose XBAR from wz block
                    wt = wpool.tile([KW, KW], bf16, tag="w")
                    nc.sync.dma_start(
                        wt[:], bass.AP(wz_t, b * WBLK, [[KW, KW], [1, KW]]),
                        transpose=True)
                    pt = pspool.tile([BLK, D], f32, tag="mm")
                    nc.tensor.matmul(pt[:], wt[:, :BLK], x_all[:, b, :])
                    evict[b % 2](out_all[:BLK, b, :], pt[:])
                nc.scalar.dma_start(
                    bass.AP(out_t, g * GB * BLK * D,
                            [[D, BLK], [BLK * D, GB], [1, D]]),
                    out_all[:BLK, g * GB:(g + 1) * GB, :])
    nc.compile()
    return nc


def _get_nc():
    if "nc" not in _CACHE:
        _CACHE["nc"] = _build()
    return _CACHE["nc"]


def _make_in_maps(x, delta_times, valid_mask, w, beta):
    w64 = w.astype(np.float64)
    wsm = np.exp(w64 - w64.max())
    wsm /= wsm.sum()
    b = 1.0 / (1.0 + np.exp(-float(beta[0])))
    bwsm = np.tile((b / (1.0 - b) * wsm)[None, :], (128, 1)).astype(np.float32)
    wz = np.zeros(NB * WBLK, np.float32).astype(ml_dtypes.bfloat16)

    in_maps = []
    for i in range(B):
        xp = np.zeros((NPAD, D), np.float32)
        xp[:N] = x[i]
        dtp = np.zeros(NPAD, np.float32)
        dtp[:N] = delta_times[i]
        vfp = np.zeros(NPAD, np.float32)
        vfp[:N] = valid_mask[i].astype(np.float32)
        in_maps.append({
            "x": xp.astype(ml_dtypes.bfloat16),
            "dt": dtp,
            "vf": vfp,
            "bwsm": bwsm,
            "wz": wz,
        })
    return in_maps


def _execute(in_maps, trace=False, **kw):
    nc = _get_nc()
    return run_bass_kernel_spmd(nc, in_maps, core_ids=list(range(B)),
                                trace=trace, **kw)


def kernel(x, delta_times, valid_mask, w, beta):
    in_maps = _make_in_maps(x, delta_times, valid_mask, w, beta)
    kr = _execute(in_maps, trace=False)
    outs = [kr.results[i]["out"][:N].astype(np.float32) for i in range(B)]
    return np.stack(outs, axis=0)


# revision 9
# speedup vs baseline: 2.2913x; 2.2913x over previous
"""AdaptiveTokenMixer Trainium2 kernel (8 NeuronCores, pure data parallel).

Per-core algorithm (one batch element per core), pipelined in 3 chunks:
  1. alpha stage runs in [105, 320] layout (partition q = (third h, block b),
     free = (pos i, tap p)); sliding windows are overlapping-stride AP reads
     of per-block dt/valid rows -- no PE transposes. Masked temporal-decay
     softmax blended with host-precomputed (b/(1-b))*softmax(w), renormalized.
  2. alpha (bf16) is scattered to a DRAM scratch with a skewed access pattern
     forming the banded mixing matrix rows interleaved (m*nj + j) per chunk.
  3. One DMA-transpose XBAR per chunk loads all its W[k, m] blocks into SBUF
     (free = (block, m), m contiguous); one 128x120 @ 128x256 bf16 matmul per
     block realizes the K-tap mixing exactly (PSUM f32).
  4. PSUM evicted to bf16 staging (DVE/ACT alternating); stores per chunk
     overlap the remaining matmuls.

Self-contained: hardcodes shapes for B=8, N=4096, d=256, K=8.
"""
import numpy as np
import ml_dtypes

import concourse.bass as bass
import concourse.bacc as bacc
import concourse.mybir as mybir
from concourse import tile
from concourse.bass_utils import run_bass_kernel_spmd

B, N, D, K = 8, 4096, 256, 8
BLK = 120                      # output positions per block
NB = (N + BLK - 1) // BLK      # 35 blocks -> covers 4200 positions
NOUT = NB * BLK                # 4200 rows in padded device output
NPAD = 4224                    # padded input length
KW = 128                       # k-window (contraction) per block
NH = 3                         # thirds per block in alpha layout
IH = BLK // NH                 # 40 positions per third
QP = NH * NB                   # 105 partitions used in alpha stage
FA = IH * K                    # 320 free elements per alpha partition
ROWW = IH + K                  # 48: dt/vf row width per third
CHUNKS = [(0, 6), (6, 14), (20, 15)]   # (first block, nblocks) per chunk
SCHUNKS = [(0, 12), (12, 12), (24, 11)]  # out-store chunking

_CACHE = {}


def _build():
    nc = bacc.Bacc("TRN2", target_bir_lowering=False, debug=False,
                   num_devices=B)
    f32 = mybir.dt.float32
    bf16 = mybir.dt.bfloat16

    x_t = nc.dram_tensor("x", [NPAD, D], bf16, kind="ExternalInput")
    dt_t = nc.dram_tensor("dt", [NPAD], f32, kind="ExternalInput")
    vf_t = nc.dram_tensor("vf", [NPAD], f32, kind="ExternalInput")
    bwsm_t = nc.dram_tensor("bwsm", [128, K], f32, kind="ExternalInput")
    wz_t = nc.dram_tensor("wz", [NB * KW * KW], bf16, kind="ExternalInput")
    out_t = nc.dram_tensor("out", [NOUT, D], bf16, kind="ExternalOutput")

    def shift(t, p0):  # [QP, (i, p)] read of t[q, i+p] (overlapping strides)
        return bass.AP(t.tensor, t.offset + p0, [t.ap[0], [1, IH], [1, K]])

    def base(t):  # [QP, (i, p-rep)] read of t[q, i]
        return bass.AP(t.tensor, t.offset, [t.ap[0], [1, IH], [0, K]])

    def pb(t):  # [QP, 320] tile -> [QP, i, p] view (p innermost, for reduce)
        return bass.AP(t.tensor, t.offset, [t.ap[0], [K, IH], [1, K]])

    def exp_i(t):  # [QP, IH] tile -> [QP, (i, p-rep)]
        return bass.AP(t.tensor, t.offset, [t.ap[0], [1, IH], [0, K]])

    def bw_rep(t):  # [QP, K] tile -> [QP, (i-rep, p)]
        return bass.AP(t.tensor, t.offset, [t.ap[0], [0, IH], [1, K]])

    with tile.TileContext(nc) as tc:
        with tc.tile_pool(name="alph", bufs=1) as apool, \
             tc.tile_pool(name="big", bufs=1) as bpool, \
             tc.tile_pool(name="ps", bufs=4, space="PSUM") as pspool:

            # ---- input loads ----
            bwsm = apool.tile([QP, K], f32)
            nc.scalar.dma_start(bwsm[:], bass.AP(bwsm_t, 0, [[K, QP], [1, K]]))
            dt_rows = apool.tile([QP, ROWW], f32)
            vf_rows = apool.tile([QP, ROWW], f32)
            for src_t, dst in ((dt_t, dt_rows), (vf_t, vf_rows)):
                nc.scalar.dma_start(
                    dst[:],
                    bass.AP(src_t, 0, [[IH, NH], [BLK, NB], [1, ROWW]]))
            # x windows per chunk: x_all[i, b, d] = x[b*120+i, d]
            x_all = bpool.tile([128, NB, D], bf16)
            for j0, nj in CHUNKS:
                nc.sync.dma_start(
                    x_all[:, j0:j0 + nj, :],
                    bass.AP(x_t, j0 * BLK * D,
                            [[D, 128], [BLK * D, nj], [1, D]]))

            # ---- alpha stage on [QP, FA] (all Vector + one ACT Exp) ----
            td = apool.tile([QP, FA], f32)
            nc.vector.tensor_tensor(td[:], shift(dt_rows, 0), base(dt_rows),
                                    mybir.AluOpType.subtract)
            nc.vector.tensor_scalar(td[:], td[:], 0.0, None,
                                    mybir.AluOpType.max)
            cv = apool.tile([QP, FA], f32)
            nc.vector.tensor_tensor(cv[:], shift(vf_rows, 0), base(vf_rows),
                                    mybir.AluOpType.mult)
            e2 = apool.tile([QP, FA], f32)
            nc.scalar.activation(e2[:], td[:],
                                 mybir.ActivationFunctionType.Exp,
                                 scale=-1.0)
            e = apool.tile([QP, FA], f32)
            nc.vector.tensor_tensor(e[:], e2[:], cv[:], mybir.AluOpType.mult)
            s = apool.tile([QP, IH], f32)
            nc.vector.tensor_reduce(s[:], pb(e), mybir.AxisListType.X,
                                    mybir.AluOpType.add)
            nc.vector.tensor_scalar(s[:], s[:], 1e-30, None,
                                    mybir.AluOpType.max)
            rcp = apool.tile([QP, IH], f32)
            nc.vector.reciprocal(rcp[:], s[:])
            au = apool.tile([QP, FA], f32)
            nc.vector.tensor_tensor(au[:], e2[:], exp_i(rcp[:, :]),
                                    mybir.AluOpType.mult)
            nc.vector.tensor_tensor(au[:], au[:], bw_rep(bwsm[:, :]),
                                    mybir.AluOpType.add)
            nc.vector.tensor_tensor(au[:], au[:], cv[:],
                                    mybir.AluOpType.mult)
            sa = apool.tile([QP, IH], f32)
            nc.vector.tensor_reduce(sa[:], pb(au), mybir.AxisListType.X,
                                    mybir.AluOpType.add)
            nc.vector.tensor_scalar(sa[:], sa[:], 1e-8, None,
                                    mybir.AluOpType.max)
            r = apool.tile([QP, IH], f32)
            nc.vector.reciprocal(r[:], sa[:])
            nc.vector.tensor_tensor(
                r[:], r[:],
                bass.AP(vf_rows.tensor, vf_rows.offset,
                        [vf_rows.ap[0], [1, IH]]),
                mybir.AluOpType.mult)
            af = apool.tile([QP, FA], bf16)
            nc.vector.tensor_tensor(af[:], au[:], exp_i(r[:, :]),
                                    mybir.AluOpType.mult)

            # ---- pipeline per chunk: skew -> batched xbar -> matmuls ----
            out_all = bpool.tile([128, NB, D], bf16)
            w_all = bpool.tile([128, NB, KW], bf16)
            evict = [nc.vector.tensor_copy, nc.scalar.copy]
            WBLK = KW * KW
            for j0, nj in CHUNKS:
                # skew write: wz[b*WBLK + m*129 + p], m = h*40 + i (block-major)
                for h in range(NH):
                    v = af[h * NB + j0: h * NB + j0 + nj]
                    nc.scalar.dma_start(
                        bass.AP(wz_t, j0 * WBLK + h * IH * (KW + 1),
                                [[WBLK, nj], [KW + 1, IH], [1, K]]),
                        bass.AP(v.tensor, v.offset, [v.ap[0], [K, IH], [1, K]]))
                # batched XBAR: w_all[k, j0+jj, m] = wz[(j0+jj)*WBLK + m*128 + k]
                nc.sync.dma_start(
                    w_all[:, j0:j0 + nj, :],
                    bass.AP(wz_t, j0 * WBLK, [[KW, nj * KW], [1, KW]]),
                    transpose=True)
                for jj in range(nj):
                    b = j0 + jj
                    pt = pspool.tile([BLK, D], f32, tag="mm")
                    nc.tensor.matmul(pt[:], w_all[:, b, :BLK], x_all[:, b, :])
                    evict[b % 2](out_all[:BLK, b, :], pt[:])
            for j0, nj in SCHUNKS:
                nc.scalar.dma_start(
                    bass.AP(out_t, j0 * BLK * D,
                            [[D, BLK], [BLK * D, nj], [1, D]]),
                    out_all[:BLK, j0:j0 + nj, :])
    nc.compile()
    return nc


def _get_nc():
    if "nc" not in _CACHE:
        _CACHE["nc"] = _build()
    return _CACHE["nc"]


def _make_in_maps(x, delta_times, valid_mask, w, beta):
    w64 = w.astype(np.float64)
    wsm = np.exp(w64 - w64.max())
    wsm /= wsm.sum()
    b = 1.0 / (1.0 + np.exp(-float(beta[0])))
    bwsm = np.tile((b / (1.0 - b) * wsm)[None, :], (128, 1)).astype(np.float32)
    wz = np.zeros(NB * KW * KW, np.float32).astype(ml_dtypes.bfloat16)

    in_maps = []
    for i in range(B):
        xp = np.zeros((NPAD, D), np.float32)
        xp[:N] = x[i]
        dtp = np.zeros(NPAD, np.float32)
        dtp[:N] = delta_times[i]
        vfp = np.zeros(NPAD, np.float32)
        vfp[:N] = valid_mask[i].astype(np.float32)
        in_maps.append({
            "x": xp.astype(ml_dtypes.bfloat16),
            "dt": dtp,
            "vf": vfp,
            "bwsm": bwsm,
            "wz": wz,
        })
    return in_maps


def _execute(in_maps, trace=False, **kw):
    nc = _get_nc()
    return run_bass_kernel_spmd(nc, in_maps, core_ids=list(range(B)),
                                trace=trace, **kw)


def kernel(x, delta_times, valid_mask, w, beta):
    in_maps = _make_in_maps(x, delta_times, valid_mask, w, beta)
    kr = _execute(in_maps, trace=False)
    outs = [kr.results[i]["out"][:N].astype(np.float32) for i in range(B)]
    return np.stack(outs, axis=0)


# revision 15
# speedup vs baseline: 2.8198x; 1.2306x over previous
"""AdaptiveTokenMixer Trainium2 kernel (8 NeuronCores, pure data parallel).

Per-core algorithm (one batch element per core), pipelined in 5 chunks:
  1. alpha stage runs in [105, 320] layout (partition q = chunk-contiguous
     (block b, third h), free = (pos i, tap p)); sliding windows are
     overlapping-stride AP reads of a single packed dt/valid/bw row tensor --
     no PE transposes. exp(-td-12) temporal-decay softmax (bias keeps the
     unmasked pad taps finite; the constant cancels in the normalization),
     blended with host-precomputed (b/(1-b))*softmax(w), renormalized.
  2. One skewed DMA per chunk scatters alpha (bf16) into a DRAM scratch
     forming banded W^T[m, k] = alpha[n0+m, k-m] per 120-position block.
  3. One DMA-transpose XBAR per chunk loads its W[k, m] blocks into SBUF;
     one 128x120 @ 128x256 bf16 matmul per block realizes the K-tap mixing
     exactly (PSUM f32, two blocks share a PSUM bank).
  4. Paired PSUM evictions to bf16 staging (DVE/ACT alternating); stores per
     7-block group overlap the remaining matmuls. Skew+XBAR share the Sync
     queue (FIFO ordering); x/alpha loads ride the Activation queue.

Self-contained: hardcodes shapes for B=8, N=4096, d=256, K=8.
"""
import numpy as np
import ml_dtypes

import concourse.bass as bass
import concourse.bacc as bacc
import concourse.mybir as mybir
from concourse import tile
from concourse.bass_utils import run_bass_kernel_spmd

B, N, D, K = 8, 4096, 256, 8
BLK = 120                      # output positions per block
NB = (N + BLK - 1) // BLK      # 35 blocks -> covers 4200 positions
NOUT = NB * BLK                # 4200 rows in padded device output
NPAD = 4224                    # padded input length
KW = 128                       # k-window (contraction) per block
WBLK = KW * KW                 # W scratch elements per block
NH = 3                         # thirds per block in alpha layout
IH = BLK // NH                 # 40 positions per third
QP = NH * NB                   # 105 partitions used in alpha stage
FA = IH * K                    # 320 free elements per alpha partition
ROWW = IH + K                  # 48: dt/vf row width per third
CW = 2 * ROWW + K + 1          # 105: packed comb row (dt | vf | bw | ebias)
CHUNKS = [(0, 5), (5, 10), (15, 10), (25, 10)]
XCHUNKS = [(0, 18), (18, 17)]            # x load chunking (one per queue)
SCHUNKS = [(0, 12), (12, 12), (24, 11)]  # out-store chunking
EBIAS = -12.0                  # exp bias: cancels in softmax, avoids overflow

_CACHE = {}


def _build():
    nc = bacc.Bacc("TRN2", target_bir_lowering=False, debug=False,
                   num_devices=B)
    f32 = mybir.dt.float32
    bf16 = mybir.dt.bfloat16

    x_t = nc.dram_tensor("x", [NPAD, D], bf16, kind="ExternalInput")
    comb_t = nc.dram_tensor("comb", [QP, CW], f32, kind="ExternalInput")
    wz_t = nc.dram_tensor("wz", [NB * WBLK], bf16, kind="ExternalInput")
    out_t = nc.dram_tensor("out", [NOUT, D], bf16, kind="ExternalOutput")

    def shift(t, c0):  # [QP, (i, p)] read of t[q, c0+i+p] (overlapping)
        return bass.AP(t.tensor, t.offset + c0, [t.ap[0], [1, IH], [1, K]])

    def base(t, c0):  # [QP, (i, p-rep)] read of t[q, c0+i]
        return bass.AP(t.tensor, t.offset + c0, [t.ap[0], [1, IH], [0, K]])

    def pb(t):  # [QP, 320] tile -> [QP, i, p] view (p innermost, for reduce)
        return bass.AP(t.tensor, t.offset, [t.ap[0], [K, IH], [1, K]])

    def exp_i(t):  # [QP, IH] tile -> [QP, (i, p-rep)]
        return bass.AP(t.tensor, t.offset, [t.ap[0], [1, IH], [0, K]])

    def bw_rep(t):  # comb bw cols -> [QP, (i-rep, p)]
        return bass.AP(t.tensor, t.offset + 2 * ROWW, [t.ap[0], [0, IH], [1, K]])

    with tile.TileContext(nc) as tc:
        with tc.tile_pool(name="alph", bufs=1) as apool, \
             tc.tile_pool(name="big", bufs=1) as bpool, \
             tc.tile_pool(name="ps", bufs=3, space="PSUM") as pspool:

            # ---- input loads (Activation queue; comb first) ----
            comb = apool.tile([QP, CW], f32)
            nc.scalar.dma_start(comb[:], bass.AP(comb_t, 0, [[CW, QP], [1, CW]]))
            x_all = bpool.tile([128, NB, D], bf16)
            for qe, (j0, nj) in zip((nc.sync, nc.scalar), XCHUNKS):
                qe.dma_start(
                    x_all[:, j0:j0 + nj, :],
                    bass.AP(x_t, j0 * BLK * D,
                            [[D, 128], [BLK * D, nj], [1, D]]))

            # ---- alpha stage on [QP, FA] (Vector + one ACT Exp) ----
            td = apool.tile([QP, FA], f32)
            nc.vector.tensor_tensor(td[:], shift(comb, 0), base(comb, 0),
                                    mybir.AluOpType.subtract)
            cv = apool.tile([QP, FA], f32)
            nc.vector.tensor_tensor(cv[:], shift(comb, ROWW),
                                    base(comb, ROWW), mybir.AluOpType.mult)
            e2 = apool.tile([QP, FA], f32)
            nc.scalar.activation(e2[:], td[:],
                                 mybir.ActivationFunctionType.Exp,
                                 bias=comb[:, CW - 1:CW], scale=-1.0)
            e = apool.tile([QP, FA], f32)
            nc.vector.tensor_tensor(e[:], e2[:], cv[:], mybir.AluOpType.mult)
            s = apool.tile([QP, IH], f32)
            nc.vector.tensor_reduce(s[:], pb(e), mybir.AxisListType.X,
                                    mybir.AluOpType.add)
            nc.vector.tensor_scalar(s[:], s[:], 1e-30, None,
                                    mybir.AluOpType.max)
            rcp = apool.tile([QP, IH], f32)
            nc.vector.reciprocal(rcp[:], s[:])
            au = apool.tile([QP, FA], f32)
            nc.vector.tensor_tensor(au[:], e[:], exp_i(rcp[:, :]),
                                    mybir.AluOpType.mult)
            nc.vector.tensor_tensor(au[:], au[:], bw_rep(comb),
                                    mybir.AluOpType.add)
            nc.vector.tensor_tensor(au[:], au[:], cv[:],
                                    mybir.AluOpType.mult)
            sa = apool.tile([QP, IH], f32)
            nc.vector.tensor_reduce(sa[:], pb(au), mybir.AxisListType.X,
                                    mybir.AluOpType.add)
            nc.vector.tensor_scalar(sa[:], sa[:], 1e-8, None,
                                    mybir.AluOpType.max)
            r = apool.tile([QP, IH], f32)
            nc.vector.reciprocal(r[:], sa[:])
            af = apool.tile([QP, FA], bf16)
            nc.vector.tensor_tensor(af[:], au[:], exp_i(r[:, :]),
                                    mybir.AluOpType.mult)

            # ---- pipeline per chunk: skew -> batched xbar -> matmuls ----
            # (skew + xbar share the Sync queue: FIFO gives cheap ordering)
            out_all = bpool.tile([128, NB, D], bf16)
            w_all = bpool.tile([128, NB, KW], bf16)
            evict = [nc.vector.tensor_copy, nc.scalar.copy]
            pts = {}

            def skew(ci):
                j0, nj = CHUNKS[ci]
                qe = nc.sync if ci % 2 == 0 else nc.scalar
                for h in range(NH):
                    v = af[h * NB + j0: h * NB + j0 + nj]
                    qe.dma_start(
                        bass.AP(wz_t, j0 * WBLK + h * IH * (KW + 1),
                                [[WBLK, nj], [KW + 1, IH], [1, K]]),
                        bass.AP(v.tensor, v.offset,
                                [v.ap[0], [K, IH], [1, K]]))

            def xbar(ci):
                j0, nj = CHUNKS[ci]
                nc.sync.dma_start(
                    w_all[:, j0:j0 + nj, :],
                    bass.AP(wz_t, j0 * WBLK, [[KW, nj * KW], [1, KW]]),
                    transpose=True)

            skew(1)
            skew(3)
            for ci, (j0, nj) in enumerate(CHUNKS):
                if ci % 2 == 0:
                    skew(ci)
                xbar(ci)
                for jj in range(nj):
                    b = j0 + jj
                    pi = b // 2
                    if b % 2 == 0:
                        pt = pspool.tile([BLK, 2, D], f32, tag="mm",
                                         name=f"pt{pi}")
                        pts[pi] = pt
                    pt = pts[pi]
                    nc.tensor.matmul(pt[:, b % 2, :], w_all[:, b, :BLK],
                                     x_all[:, b, :])
                    if b % 2 == 1:
                        evict[pi % 2](out_all[:BLK, b - 1:b + 1, :], pt[:])
                    elif b == NB - 1:
                        evict[pi % 2](out_all[:BLK, b, :], pt[:, 0, :])
            for j0, nj in SCHUNKS:
                nc.sync.dma_start(
                    bass.AP(out_t, j0 * BLK * D,
                            [[D, BLK], [BLK * D, nj], [1, D]]),
                    out_all[:BLK, j0:j0 + nj, :])
    nc.compile()
    return nc


def _get_nc():
    if "nc" not in _CACHE:
        _CACHE["nc"] = _build()
    return _CACHE["nc"]


def _make_in_maps(x, delta_times, valid_mask, w, beta):
    w64 = w.astype(np.float64)
    wsm = np.exp(w64 - w64.max())
    wsm /= wsm.sum()
    b = 1.0 / (1.0 + np.exp(-float(beta[0])))
    bw = (b / (1.0 - b) * wsm).astype(np.float32)
    wz = np.zeros(NB * WBLK, np.float32).astype(ml_dtypes.bfloat16)

    in_maps = []
    for i in range(B):
        xp = np.zeros((NPAD, D), np.float32)
        xp[:N] = x[i]
        dtp = np.zeros(NPAD, np.float32)
        dtp[:N] = delta_times[i]
        vfp = np.zeros(NPAD, np.float32)
        vfp[:N] = valid_mask[i].astype(np.float32)
        comb = np.zeros((QP, CW), np.float32)
        for h in range(NH):
            for bb in range(NB):
                q = h * NB + bb
                o = bb * BLK + h * IH
                comb[q, 0:ROWW] = dtp[o:o + ROWW]
                comb[q, ROWW:2 * ROWW] = vfp[o:o + ROWW]
                comb[q, 2 * ROWW:2 * ROWW + K] = bw
                comb[q, CW - 1] = EBIAS
        in_maps.append({
            "x": xp.astype(ml_dtypes.bfloat16),
            "comb": comb,
            "wz": wz,
        })
    return in_maps


def _execute(in_maps, trace=False, **kw):
    nc = _get_nc()
    return run_bass_kernel_spmd(nc, in_maps, core_ids=list(range(B)),
                                trace=trace, **kw)


def kernel(x, delta_times, valid_mask, w, beta):
    in_maps = _make_in_maps(x, delta_times, valid_mask, w, beta)
    kr = _execute(in_maps, trace=False)
    outs = [kr.results[i]["out"][:N].astype(np.float32) for i in range(B)]
    return np.stack(outs, axis=0)
